# revision 58
# baseline (speedup 1.0000x reference)
"""Multi-head attention (B=2, S=2048, D=1024, H=16) on 8 Trainium2 NeuronCores.

Sharding: core c -> batch b = c // 4, head-group g = c % 4 (4 heads = 256 proj
dims per core). Each core computes its 4 heads' attention plus the matching
slice of the output projection; the host sums the 4 partial outputs per batch
and adds bo.

v8 changes vs v2 (221us -> ~211us):
  - kc processed in PAIRS with per-hh [128,512] score tiles (p1 bufs=4):
    each QK's PSUM buffer is freed two kc earlier, so the scheduler keeps
    all four row-tiled QK matmuls back-to-back -- one PE array-mode switch
    per direction per kc-pair instead of per kc (each switch drains the
    array, ~95ns).
  - P@V stationary widened to [128,128]: cols 64-127 are ones, so PSUM rows
    64-127 hold the softmax denominator REPLICATED 64x. Normalize becomes
    copy+reciprocal([64,512])+fused mul -- no 1-partition ops, no gpsimd
    partition_broadcast (~6us critical latency -> ~2.5us). The copy is
    needed: the custom-DVE reciprocal misreads shifted partition bases on
    HW (CoreSim disagrees).
  - K projection in fp8e4m3 DoubleRow (K=256/instruction, half the PE
    stream cycles). Host pre-scales Wk by 64 to dodge fp8 subnormals; the
    bias activation rescales by 1/64. rel err 7.8e-3 -> 1.39e-2 (gate
    2e-2). q-projection fp8 as well was tried: no wall-clock gain (extra
    DR<->normal mode switches) and worse error -- reverted.
  - bv folded into bo on the host: sum(p)/D == 1 makes the v-bias a
    constant shift of attn_out, which commutes through Wo.
  - lazy start: only ws_k + kt quarter 0 + ws_q + qt s0 (head-pair 0) gate
    the first QK (~23us vs ~35us); remaining kT/qT/v chains stream from
    inside qb0-pair0 just ahead of their consumers.
  - exp split ACT:DVE = 10:6 per 16 kc, spread so pair tails aren't
    ACT-only (bunching stalled QK on score-buffer recycle).
  - deeper PV pipeline (pend>4), early drain on the last pair; final-emit
    c0 matmuls issued before the last normalize; tail copies alternate
    DVE/ACT.
"""

import ml_dtypes
import numpy as np

import concourse.bass as bass
import concourse.mybir as mybir
import concourse.tile as tile
from concourse import bacc
from concourse.bass_utils import run_bass_kernel_spmd

B, S, D, H = 2, 2048, 1024, 16
OL = 256          # local projection dims (4 heads x 64)
NI = D // 128     # contraction chunks for projections
NK = S // 128     # key chunks
NQ = S // 512     # query blocks

# kc tiles whose exp runs whole-tile on VectorE (Schraudolph: one
# tensor_scalar mult+add writing int16 bits that bitcast to fp16) instead of
# ScalarE's native Exp; one per kc-pair for the first six pairs
DVE_KC = (1, 3, 7, 9, 13, 15)
# exp(0.125*x) ~= bitcast_fp16(int16(EXP_A*x + EXP_B))
EXP_A = 0.125 * 1.4426950408889634 * 1024.0
EXP_B = 15360.0 - 44.7

# qb0-pair0 v_chain schedule: kc2 -> v projection chunks, placed just
# behind the vt DMA arrivals and just ahead of the consuming PV matmuls
VCHAIN_SCHED = {1: (0, 1), 2: (2, 3), 3: (4, 5, 6), 4: (7, 8),
                5: (9, 10, 11), 6: (12, 13), 7: (14, 15)}

_CACHE = {}


def _build():
    DT = mybir.dt.float16
    F8 = mybir.dt.float8e4
    F32 = mybir.dt.float32
    I16 = mybir.dt.int16
    AF = mybir.ActivationFunctionType
    MUL = mybir.AluOpType.mult
    ADD = mybir.AluOpType.add
    DR = mybir.MatmulPerfMode.DoubleRow

    nc = bacc.Bacc("TRN2", target_bir_lowering=False, debug=False, num_devices=8)

    qt_d = nc.dram_tensor("qt", [D, S], DT, kind="ExternalInput").ap() \
        .rearrange("(c p) s -> p c s", p=128)
    kt_d = nc.dram_tensor("kt", [D, S], F8, kind="ExternalInput").ap() \
        .rearrange("(c p) s -> p c s", p=128)
    vt_d = nc.dram_tensor("vt", [D, S], DT, kind="ExternalInput").ap() \
        .rearrange("(c p) s -> p c s", p=128)
    wq_d = nc.dram_tensor("wqt", [D, OL], DT, kind="ExternalInput").ap() \
        .rearrange("(c p) o -> p c o", p=128)
    wk_d = nc.dram_tensor("wkt", [D, OL], F8, kind="ExternalInput").ap() \
        .rearrange("(c p) o -> p c o", p=128)
    wv_d = nc.dram_tensor("wvt", [D, OL], DT, kind="ExternalInput").ap() \
        .rearrange("(c p) o -> p c o", p=128)
    bq_d = nc.dram_tensor("bq2", [2, 128, 1], F32, kind="ExternalInput").ap()
    bk_d = nc.dram_tensor("bk2", [2, 128, 1], F32, kind="ExternalInput").ap()
    wo_d = nc.dram_tensor("wot", [OL, D], DT, kind="ExternalInput").ap() \
        .rearrange("(c p) o -> p c o", p=128)
    out_d = nc.dram_tensor("out_t", [D, S], DT, kind="ExternalOutput").ap() \
        .rearrange("(c p) s -> c p s", p=128)

    with tile.TileContext(nc) as tc:
        with (
            tc.tile_pool(name="per", bufs=1) as per,
            tc.tile_pool(name="pr", bufs=8) as pr,
            tc.tile_pool(name="sm", bufs=2) as sm,
            tc.tile_pool(name="ot", bufs=2) as ot,
            tc.tile_pool(name="osg", bufs=4) as osg,
            tc.tile_pool(name="pj", bufs=2, space="PSUM") as pj,
            tc.tile_pool(name="p1", bufs=4, space="PSUM") as p1,
            tc.tile_pool(name="px", bufs=2, space="PSUM") as px,
        ):
            # --- persistent tiles
            as_k = per.tile([128, NI, S], F8, tag="ak", name="ak")
            as_q = per.tile([128, NI, S], DT, tag="aq", name="aq")
            as_v = per.tile([128, NI, S], DT, tag="av", name="av")
            ws_k = per.tile([128, NI, OL], F8, tag="wk", name="wk")
            ws_q = per.tile([128, NI, OL], DT, tag="wq", name="wq")
            ws_v = per.tile([128, NI, OL], DT, tag="wv", name="wv")
            wo_sb = per.tile([128, 2, D], DT, tag="wo", name="wo")
            qt_sb = [per.tile([128, S], DT, tag=f"qt{m}", name=f"qt{m}")
                     for m in range(2)]
            kt_sb = [per.tile([128, S], DT, tag=f"kt{m}", name=f"kt{m}")
                     for m in range(2)]
            # [keys, head, 64 v-dims + 64 ones]; the ones columns make PSUM
            # rows 64-127 of the PV accumulation hold the softmax denominator
            # replicated across 64 partitions (vectorizes the normalize)
            v_sb = [per.tile([128, 4, 128], DT, tag=f"v{sc}", name=f"v{sc}")
                    for sc in range(NK)]
            bq_sb = [per.tile([128, 1], F32, tag=f"bq{m}", name=f"bq{m}")
                     for m in range(2)]
            bk_sb = [per.tile([128, 1], F32, tag=f"bk{m}", name=f"bk{m}")
                     for m in range(2)]

            # ones regions filled by DVE memsets in the prefix (off the
            # critical path); disjoint from the ACT v-copy region so no
            # false deps
            for sc in range(NK):
                nc.vector.memset(v_sb[sc][:, :, 64:128], 1.0)

            # HAM warmup: the PE idles ~6us waiting for the first kt
            # quarter; junk matmuls there lift the clock gate to 8/8 so
            # the real projection chains run at 2.4GHz instead of 1.2
            wsrc = per.tile([128, 512], DT, tag="wsrc", name="wsrc")
            nc.vector.memset(wsrc[:], 0.0)

            def warm_mms(n):
                for _ in range(n):
                    wps = pj.tile([128, 512], F32, tag="pj", name="warm")
                    nc.tensor.matmul(
                        wps[:], wsrc[:, 0:128], wsrc[:],
                        start=True, stop=True)

            # engines only come alive ~8us in; the first kt quarter lands
            # ~12us. junk MMs bridge the whole window so the real chains
            # start at 2.4GHz (and the PE never idles into a HAM throttle)
            warm_mms(36)

            # --- input loads, Sync HWDGE FIFO, strictly in consumption
            # order, 0.5-2MB per transfer. Only ws_k + kt quarter 0 +
            # ws_q + qt s0 gate the first QK (~15us); the remaining kt
            # quarters and vt stream behind, consumed by kT/v chains
            # issued from inside the qb0 loop.
            for m in range(2):
                nc.sync.dma_start(bq_sb[m][:], bq_d[m])
                nc.sync.dma_start(bk_sb[m][:], bk_d[m])
            nc.sync.dma_start(ws_k[:], wk_d)
            nc.sync.dma_start(as_k[:, :, 0:512], kt_d[:, :, 0:512])
            nc.sync.dma_start(ws_q[:], wq_d)
            nc.sync.dma_start(as_q[:, :, 0:512], qt_d[:, :, 0:512])
            nc.sync.dma_start(as_k[:, :, 512:1024], kt_d[:, :, 512:1024])
            nc.sync.dma_start(ws_v[:], wv_d)
            nc.sync.dma_start(as_v[:, :, 0:512], vt_d[:, :, 0:512])
            nc.sync.dma_start(as_k[:, :, 1024:1536], kt_d[:, :, 1024:1536])
            nc.sync.dma_start(as_v[:, :, 512:1024], vt_d[:, :, 512:1024])
            nc.sync.dma_start(as_k[:, :, 1536:2048], kt_d[:, :, 1536:2048])
            nc.sync.dma_start(wo_sb[:], wo_d)

            def dr_chain(ws, as_, bias_sb, dst_sb, m, s):
                # fp8 DoubleRow: two 128-deep contraction chunks per
                # instruction (K=256 effective) -> half the PE stream
                # cycles. Host pre-scales the weights by 64 (keeps
                # fp8e4m3 out of subnormals); the bias activation
                # rescales by 1/64.
                acc = pj.tile([128, 512], F32, tag="pj", name="pj")
                for p in range(NI // 2):
                    nc.tensor.matmul(
                        acc[:],
                        ws[:, 2 * p:2 * p + 2, m * 128:(m + 1) * 128],
                        as_[:, 2 * p:2 * p + 2, s * 512:(s + 1) * 512],
                        start=(p == 0),
                        stop=(p == NI // 2 - 1),
                        perf_mode=DR,
                    )
                nc.scalar.activation(
                    dst_sb[m][:, s * 512:(s + 1) * 512], acc[:],
                    AF.Identity, bias=bias_sb[m][:], scale=1.0 / 64.0)

            def q_chain(m, s):
                acc = pj.tile([128, 512], F32, tag="pj", name="pj")
                for i in range(NI):
                    nc.tensor.matmul(
                        acc[:],
                        ws_q[:, i, m * 128:(m + 1) * 128],
                        as_q[:, i, s * 512:(s + 1) * 512],
                        start=(i == 0),
                        stop=(i == NI - 1),
                    )
                nc.scalar.activation(
                    qt_sb[m][:, s * 512:(s + 1) * 512], acc[:],
                    AF.Identity, bias=bq_sb[m][:])

            def k_chain(m, s):
                dr_chain(ws_k, as_k, bk_sb, kt_sb, m, s)

            def v_chain(sc):
                acc = pj.tile([128, OL], F32, tag="pj", name="pj")
                for i in range(NI):
                    nc.tensor.matmul(
                        acc[:],
                        as_v[:, i, sc * 128:(sc + 1) * 128],
                        ws_v[:, i, :],
                        start=(i == 0),
                        stop=(i == NI - 1),
                    )
                # ACT is stride-insensitive (1 elem/cycle); the strided
                # 4x64 dest costs DVE more but ACT only ~357ns
                nc.scalar.activation(
                    v_sb[sc][:, :, 0:64],
                    acc[:].rearrange("p (h d) -> p h d", h=4),
                    AF.Identity,
                )

            # prefix chains: ONLY what the first QK consumes -- kT sg0 and
            # qT s0 for head-pair 0. Head-pair 1 and kT sg1-3 stream from
            # inside qb0-pair0 (pair1 doesn't run for another ~15us).
            k_chain(0, 0)
            q_chain(0, 0)

            # --- attention + output projection, per query block
            def emit_op(qb_, ots_src, oc, pool, tg, on_act=False):
                osl = slice(oc * 128, (oc + 1) * 128)
                pso = pool.tile([128, 512], F32, tag=tg, name="pso")
                for c in range(2):
                    nc.tensor.matmul(
                        pso[:], wo_sb[:, c, osl], ots_src[c][:],
                        start=(c == 0), stop=(c == 1),
                    )
                st = osg.tile([128, 512], DT, tag="st", name="st")
                if on_act:
                    nc.scalar.copy(st[:], pso[:])
                else:
                    nc.vector.tensor_copy(st[:], pso[:])
                # odd stores drain on the gpsimd SWDGE ring so two rings
                # pipeline the per-DMA fixed cost (matters for the tail)
                eng = nc.sync if oc % 2 == 0 else nc.gpsimd
                eng.dma_start(
                    out_d[oc][:, qb_ * 512:(qb_ + 1) * 512], st[:])

            ots_prev = None
            for qb in range(NQ):
                qsl = slice(qb * 512, (qb + 1) * 512)
                ots = [ot.tile([128, 512], DT, tag=f"c{c}", name=f"otc{c}")
                       for c in range(2)]
                # last qb: previous block's emits spread across BOTH pairs
                # (pair1 has no q-stream work and the exp engines pace the
                # PE there); earlier qbs keep them in pair0
                qb_op_iter = iter(range(8)) if ots_prev is not None else None
                last_psos = []
                for pair in range(2):
                    acc = [px.tile([128, 512], F32, tag="x", name="acc")
                           for _ in range(2)]
                    pend = []
                    for kc2 in range(NK // 2):
                        kcs = (2 * kc2, 2 * kc2 + 1)
                        # --- row-tiled section: both kc's QK pairs, all 4
                        # matmuls back-to-back. Per-hh [128,512] score
                        # tiles (p1 bufs=4, same 4 banks) so each QK's
                        # buffer was freed two kc earlier and the
                        # scheduler can keep the row-tiled matmuls
                        # adjacent -- one array-mode switch per direction
                        # per kc pair instead of per kc.
                        ps1s = []
                        for kc in kcs:
                            ksl = slice(kc * 128, (kc + 1) * 128)
                            hts = []
                            for hh in range(2):
                                psl = slice(hh * 64, (hh + 1) * 64)
                                ps1h = p1.tile([128, 512], F32, tag="s",
                                               name="s")
                                nc.tensor.matmul(
                                    ps1h[:],
                                    kt_sb[pair][psl, ksl],
                                    qt_sb[pair][psl, qsl],
                                    start=True, stop=True,
                                )
                                hts.append(ps1h)
                            ps1s.append(hts)
                        # exp split across both engines (measured rates:
                        # ACT ~(172+n)/1.2, DVE ~(120+n)/0.96 ns)
                        for kc, hts in zip(kcs, ps1s):
                            prob = pr.tile([128, 1024], DT, tag="p", name="p")
                            for hh in range(2):
                                dst = prob[:, hh * 512:(hh + 1) * 512]
                                if kc in DVE_KC:
                                    nc.vector.tensor_scalar(
                                        out=dst.bitcast(I16), in0=hts[hh][:],
                                        scalar1=EXP_A, scalar2=EXP_B,
                                        op0=MUL, op1=ADD,
                                    )
                                else:
                                    nc.scalar.activation(
                                        dst, hts[hh][:], AF.Exp, scale=0.125
                                    )
                            pend.append((kc, prob))
                        # --- full-array section: streams, PV, emits
                        if qb == 0 and pair == 0:
                            # stream vt q2/q3, the remaining kT chains
                            # (sg1-3, both head-pairs), and the v chains,
                            # each just ahead of its consumer
                            if kc2 == 0:
                                nc.sync.dma_start(
                                    as_v[:, :, 1024:1536],
                                    vt_d[:, :, 1024:1536])
                            elif kc2 == 1:
                                nc.sync.dma_start(
                                    as_v[:, :, 1536:2048],
                                    vt_d[:, :, 1536:2048])
                            # kT chains: (m, sg) so that kt_sb[0] sg_g is
                            # ready one kc2 ahead of QK(kc=4g); two of the
                            # head-pair-1 chains move to pair1 kc2 0-1
                            # (free slots there; pair0 is the densest
                            # phase of the whole kernel)
                            KSCHED = {0: (1, 0), 1: (0, 1), 3: (0, 2),
                                      5: (0, 3), 6: (1, 3)}
                            if kc2 in KSCHED:
                                k_chain(*KSCHED[kc2])
                            if kc2 == 1:
                                q_chain(1, 0)
                            for sc in VCHAIN_SCHED.get(kc2, ()):
                                v_chain(sc)
                        if qb == 0 and pair == 1 and kc2 in (0, 1):
                            # deferred head-pair-1 kT chains (needed by
                            # this pair's QK(kc 4/8) two kc2s later; pj is
                            # free until the qacc streams start at kc2 2)
                            k_chain(1, 1 + kc2)
                        if pair == 1 and qb < NQ - 1:
                            # stream qt[s=qb+1] + its projection chains,
                            # two contraction steps (4 matmuls) per kc2
                            sN = qb + 1
                            if kc2 == 0:
                                nc.sync.dma_start(
                                    as_q[:, :, sN * 512:(sN + 1) * 512],
                                    qt_d[:, :, sN * 512:(sN + 1) * 512])
                            elif kc2 == 2:
                                qaccs = [pj.tile([128, 512], F32, tag="pj",
                                                 name="qacc")
                                         for _ in range(2)]
                            if 2 <= kc2 < 6:
                                for i in (2 * (kc2 - 2), 2 * (kc2 - 2) + 1):
                                    for m in range(2):
                                        nc.tensor.matmul(
                                            qaccs[m][:],
                                            ws_q[:, i, m * 128:(m + 1) * 128],
                                            as_q[:, i,
                                                 sN * 512:(sN + 1) * 512],
                                            start=(i == 0),
                                            stop=(i == NI - 1),
                                        )
                            elif kc2 == 6:
                                for m in range(2):
                                    nc.scalar.activation(
                                        qt_sb[m][:, sN * 512:(sN + 1) * 512],
                                        qaccs[m][:], AF.Identity,
                                        bias=bq_sb[m][:],
                                    )
                        # drain the PV pipeline early at each pair's tail
                        # so the normalize chain starts sooner (exp slack
                        # back to 1 kc2 there -- measured fine)
                        lim = 2 if kc2 >= 6 else 4
                        while len(pend) > lim:
                            pkc, pprob = pend.pop(0)
                            for hh in range(2):
                                nc.tensor.matmul(
                                    acc[hh][:],
                                    v_sb[pkc][:, pair * 2 + hh, :],
                                    pprob[:, hh * 512:(hh + 1) * 512],
                                    start=(pkc == 0), stop=(pkc == NK - 1),
                                )
                        if qb_op_iter is not None:
                            if qb < NQ - 1:
                                n_em = 2 if (pair == 0 and kc2 >= 4) else 0
                            else:
                                n_em = 1 if (pair == 0 and kc2 >= 4) or \
                                            (pair == 1 and kc2 < 4) else 0
                            for _ in range(n_em):
                                oc = next(qb_op_iter, None)
                                if oc is not None:
                                    emit_op(qb - 1, ots_prev, oc, pj, "pj")
                        if qb == NQ - 1 and pair == 1 and kc2 >= 6:
                            # fill the last pair's idle slots with the
                            # final emits' first-half matmuls (pj is free
                            # here: no qacc streams in the last qb)
                            oc = kc2 - 6
                            pso = pj.tile([128, 512], F32, tag="pj",
                                          name="pso")
                            nc.tensor.matmul(
                                pso[:], wo_sb[:, 0, oc * 128:(oc + 1) * 128],
                                ots[0][:], start=True, stop=False)
                            last_psos.append(pso)
                    # hh-major drain: acc[0] completes two matmuls earlier,
                    # so its normalize chain overlaps the hh=1 tail
                    for hh in range(2):
                        for pkc, pprob in pend:
                            nc.tensor.matmul(
                                acc[hh][:], v_sb[pkc][:, pair * 2 + hh, :],
                                pprob[:, hh * 512:(hh + 1) * 512],
                                start=(pkc == 0), stop=(pkc == NK - 1),
                            )
                    if pair == 1 and qb < NQ - 1:
                        # junk MMs keep HAM at 8/8 across the normalize
                        # latency so the next block's MMs run at full clock
                        # (skipped for the last pair: pj holds the early
                        # final-emit psos there)
                        warm_mms(2)
                    # normalize: PSUM rows 64-127 hold D replicated across
                    # 64 partitions. Copy D down to partitions 0-63 (plain
                    # copies handle the crossbar shift; the custom-DVE
                    # reciprocal does not on HW -- base-64 placement and
                    # shifted-read TT muls both produced NaN when tried),
                    # then aligned reciprocal + one fused (PSUM x rec) mul.
                    for hh in range(2):
                        dsb = sm.tile([64, 512], F32, tag=f"d{hh}",
                                      name=f"d{hh}")
                        nc.vector.tensor_copy(dsb[:], acc[hh][64:128, :])
                        rec = sm.tile([64, 512], F32, tag=f"rc{hh}",
                                      name=f"rc{hh}")
                        nc.vector.reciprocal_approx_fast(rec[:], dsb[:])
                        nc.vector.tensor_mul(
                            ots[pair][hh * 64:(hh + 1) * 64, :],
                            acc[hh][0:64, :], rec[:],
                        )
                ots_prev = ots
            # final 8 emits: oc0/oc1 c0 matmuls were issued inside the last
            # pair; oc2/oc3 c0s go on px (banks free as the normalize muls
            # retire) so the PE streams while the normalize chain finishes
            for oc in (2, 3):
                pso = px.tile([128, 512], F32, tag="x", name="pso")
                nc.tensor.matmul(
                    pso[:], wo_sb[:, 0, oc * 128:(oc + 1) * 128],
                    ots_prev[0][:], start=True, stop=False)
                last_psos.append(pso)
            for oc in range(4):
                pso = last_psos[oc]
                nc.tensor.matmul(
                    pso[:], wo_sb[:, 1, oc * 128:(oc + 1) * 128],
                    ots_prev[1][:], start=False, stop=True)
                st = osg.tile([128, 512], DT, tag="st", name="st")
                if oc % 2 == 1:
                    nc.scalar.copy(st[:], pso[:])
                else:
                    nc.vector.tensor_copy(st[:], pso[:])
                eng = nc.sync if oc % 2 == 0 else nc.gpsimd
                eng.dma_start(
                    out_d[oc][:, (NQ - 1) * 512:NQ * 512], st[:])
            for oc in range(4, 8):
                emit_op(NQ - 1, ots_prev, oc, (pj, px)[oc % 2],
                        ("pj", "x")[oc % 2], on_act=(oc % 2 == 1))

    nc.compile()
    return nc


def _get_nc():
    if "nc" not in _CACHE:
        _CACHE["nc"] = _build()
    return _CACHE["nc"]


def make_in_maps(Q, K, V, Wq, bq, Wk, bk, Wv, bv, Wo, bo):
    f = np.float32
    bf = np.float16
    in_maps = []
    for core in range(8):
        b, g = divmod(core, 4)
        sl = slice(g * OL, (g + 1) * OL)
        in_maps.append({
            "qt": np.ascontiguousarray(Q[b].T, dtype=bf),
            "kt": np.ascontiguousarray(K[b].T).astype(ml_dtypes.float8_e4m3),
            "vt": np.ascontiguousarray(V[b].T, dtype=bf),
            "wqt": np.ascontiguousarray(Wq[sl].T, dtype=bf),
            "wkt": (np.ascontiguousarray(Wk[sl].T) * 64.0
                    ).astype(ml_dtypes.float8_e4m3),
            "wvt": np.ascontiguousarray(Wv[sl].T, dtype=bf),
            "bq2": np.ascontiguousarray(bq[sl].reshape(2, 128, 1), dtype=f),
            "bk2": np.ascontiguousarray(bk[sl].reshape(2, 128, 1), dtype=f),
            "wot": np.ascontiguousarray(Wo[:, sl].T, dtype=bf),
        })
    return in_maps


def kernel(Q, K, V, Wq, bq, Wk, bk, Wv, bv, Wo, bo):
    nc = _get_nc()
    in_maps = make_in_maps(Q, K, V, Wq, bq, Wk, bk, Wv, bv, Wo, bo)
    res = run_bass_kernel_spmd(nc, in_maps, core_ids=list(range(8)))
    # sum(p)/D == 1 makes the v-bias a constant shift of attn_out, which
    # commutes through the output projection: fold bv into bo here.
    bo_eff = bo + Wo @ bv
    out = np.empty((B, S, D), np.float32)
    for b in range(B):
        acc = res.results[b * 4 + 0]["out_t"].astype(np.float32)
        for g in range(1, 4):
            acc += res.results[b * 4 + g]["out_t"]
        out[b] = (acc.T + bo_eff).astype(np.float32)
    return out
